# revision 2
# baseline (speedup 1.0000x reference)
"""Trainium2 Bass kernel for nn_ContrastiveEmbeddingLoss.

Reference computation (N=8192, D=128, margin=1.0):
    d[i,j]  = ||x_i - x_j||^2          (clamped at 0)
    same    = (y_i == y_j)
    loss    = mean((1-same)*d + same*relu(margin - d))

Algebraic decomposition used here:
    loss_sum = sum_ij d  -  sum_same d  +  sum_same relu(1 - d)

The first two terms are exact O(N*D) sums-of-moments:
    sum_ij d   = 2*N*sum(sq) - 2*||sum_i x_i||^2
    sum_same d = sum_c [ 2*n_c*sum_{i in c} sq_i - 2*||sum_{i in c} x_i||^2 ]
computed on host in float64 (tiny next to the device work and *more*
accurate than the reference's own fp32 mean over 67M elements).

Only sum_same relu(1 - d) needs pairwise work, and `same` is nonzero only
within a class. The host sorts rows by class into 128-wide slots
(13 slots/core x 8 cores = 104 >= number of classes), and each core runs:

    psum = X_s^T X_s (K=128, scaled by sqrt(2)) + L_s^T R_s (K=3 augment)

with L = [valid; 2*sq; valid], R = [-sq; -valid/2; valid] so that for a
real-real pair psum = 1 - d exactly, and any pair involving a padding
column gives psum = 0. A single ScalarE activation per PSUM bank then
computes relu(psum) and its per-partition running sum (accum_out), which
is the entire hinge contribution -- no masks, no pad corrections.

Device traffic per core: 1/8th of x in bf16 (~426 KB) + two 10 KB side
tensors; 26 small matmuls; 4 activations; no DVE work.
"""

import numpy as np
import ml_dtypes

N, D = 8192, 128
MARGIN = 1.0
NCORES = 8
SLOT = 128               # columns per class slot (max class size supported)
SLOTS_PER_CORE = 13      # 8*13 = 104 slots >= 100 classes
W = SLOTS_PER_CORE * SLOT
# bank groups of slots: each group's matmuls share one PSUM bank (<=512 free)
GROUPS = [(0, 4), (4, 4), (8, 4), (12, 1)]

_BF16 = ml_dtypes.bfloat16
_NC = None


def _build_nc():
    import concourse.bacc as bacc
    import concourse.mybir as mybir
    import concourse.tile as tile

    nc = bacc.Bacc(None, target_bir_lowering=False)

    x1 = nc.declare_dram_parameter("x1", [D, W], mybir.dt.bfloat16, isOutput=False)
    l2 = nc.declare_dram_parameter("l2", [3, W], mybir.dt.bfloat16, isOutput=False)
    r2 = nc.declare_dram_parameter("r2", [3, W], mybir.dt.bfloat16, isOutput=False)
    acc = nc.declare_dram_parameter(
        "acc", [D, len(GROUPS)], mybir.dt.float32, isOutput=True
    )

    with tile.TileContext(nc) as tc:
        with (
            tc.tile_pool(name="sb", bufs=1) as sb,
            tc.tile_pool(name="ps", bufs=4, space="PSUM") as ps,
            tc.tile_pool(name="v", bufs=2) as vp,
        ):
            x1t = sb.tile([D, W], mybir.dt.bfloat16)
            nc.sync.dma_start(x1t[:], x1[:])
            l2t = sb.tile([3, W], mybir.dt.bfloat16)
            nc.sync.dma_start(l2t[:], l2[:])
            r2t = sb.tile([3, W], mybir.dt.bfloat16)
            nc.sync.dma_start(r2t[:], r2[:])
            accst = sb.tile([D, len(GROUPS)], mybir.dt.float32)

            for g, (s0, nslots) in enumerate(GROUPS):
                gw = nslots * SLOT
                psum = ps.tile([128, gw], mybir.dt.float32, tag="psum")
                for k in range(nslots):
                    s = s0 + k
                    cols = slice(s * SLOT, (s + 1) * SLOT)
                    pcols = slice(k * SLOT, (k + 1) * SLOT)
                    nc.tensor.matmul(
                        psum[:, pcols], x1t[:, cols], x1t[:, cols],
                        start=True, stop=False,
                    )
                    nc.tensor.matmul(
                        psum[:, pcols], l2t[:, cols], r2t[:, cols],
                        start=False, stop=True,
                    )
                v1 = vp.tile([128, gw], mybir.dt.bfloat16, tag="v1")
                nc.scalar.activation(
                    v1[:], psum[:], mybir.ActivationFunctionType.Relu,
                    bias=0.0, scale=1.0, accum_out=accst[:, g : g + 1],
                )

            nc.sync.dma_start(acc[:], accst[:])
    nc.finalize()
    return nc


def _get_nc():
    global _NC
    if _NC is None:
        _NC = _build_nc()
    return _NC


def _prepare_inputs(x_np, y_np):
    """Host-side packing + exact fp64 moment sums.

    Returns (in_maps, sum_d_all, sum_d_same)."""
    x64 = x_np.astype(np.float64)
    sq64 = np.einsum("ij,ij->i", x64, x64)
    s_all = x64.sum(0)
    sum_d_all = 2.0 * N * sq64.sum() - 2.0 * float(s_all @ s_all)

    sq32 = sq64.astype(np.float32)
    order = np.argsort(y_np, kind="stable")
    uniq, counts = np.unique(y_np, return_counts=True)
    assert len(uniq) <= NCORES * SLOTS_PER_CORE, "too many classes for slots"
    assert counts.max() <= SLOT, "class larger than one slot"

    X1 = np.zeros((NCORES, D, W), np.float32)
    L2 = np.zeros((NCORES, 3, W), np.float32)
    R2 = np.zeros((NCORES, 3, W), np.float32)
    sum_d_same = 0.0
    pos = 0
    root2 = np.float32(np.sqrt(2.0))
    for ci, n_c in enumerate(counts):
        idx = order[pos : pos + n_c]
        pos += n_c
        core, ls = divmod(ci, SLOTS_PER_CORE)
        c0 = ls * SLOT
        X1[core][:, c0 : c0 + n_c] = (root2 * x_np[idx]).T
        L2[core][0, c0 : c0 + n_c] = 1.0
        L2[core][1, c0 : c0 + n_c] = 2.0 * sq32[idx]
        L2[core][2, c0 : c0 + n_c] = 1.0
        R2[core][0, c0 : c0 + n_c] = -sq32[idx]
        R2[core][1, c0 : c0 + n_c] = -0.5
        R2[core][2, c0 : c0 + n_c] = 1.0
        sc = x64[idx].sum(0)
        sum_d_same += 2.0 * n_c * sq64[idx].sum() - 2.0 * float(sc @ sc)

    in_maps = [
        {
            "x1": np.ascontiguousarray(X1[i]).astype(_BF16),
            "l2": np.ascontiguousarray(L2[i]).astype(_BF16),
            "r2": np.ascontiguousarray(R2[i]).astype(_BF16),
        }
        for i in range(NCORES)
    ]
    return in_maps, sum_d_all, sum_d_same


def _run_device(in_maps, trace=False):
    from concourse.bass_utils import run_bass_kernel_spmd

    return run_bass_kernel_spmd(
        _get_nc(), in_maps, core_ids=list(range(NCORES)), trace=trace
    )


def kernel(x, y):
    x_np = np.asarray(x, dtype=np.float32).reshape(N, D)
    y_np = np.asarray(y).astype(np.int64).ravel()

    in_maps, sum_d_all, sum_d_same = _prepare_inputs(x_np, y_np)
    res = _run_device(in_maps)
    hinge = sum(float(r["acc"].astype(np.float64).sum()) for r in res.results)

    loss = (sum_d_all - sum_d_same + hinge) / (float(N) * float(N))
    return np.float32(loss)


# revision 12
# speedup vs baseline: 1.0578x; 1.0578x over previous
"""Trainium2 Bass kernel for nn_ContrastiveEmbeddingLoss.

Reference computation (N=8192, D=128, margin=1.0):
    d[i,j]  = ||x_i - x_j||^2          (clamped at 0)
    same    = (y_i == y_j)
    loss    = mean((1-same)*d + same*relu(margin - d))

Algebraic decomposition used here:
    loss_sum = sum_ij d  -  sum_same d  +  sum_same relu(1 - d)

The first two terms are exact O(N*D) sums-of-moments:
    sum_ij d   = 2*N*sum(sq) - 2*||sum_i x_i||^2
    sum_same d = sum_c [ 2*n_c*sum_{i in c} sq_i - 2*||sum_{i in c} x_i||^2 ]
computed on host in float64 (tiny next to the device work and *more*
accurate than the reference's own fp32 mean over 67M elements).

Only sum_same relu(1 - d) needs pairwise work, and `same` is nonzero only
within a class. The host sorts rows by class into 128-wide slots
(13 slots/core x 8 cores = 104 >= number of classes), and each core runs:

    psum = X_s^T X_s (K=128, scaled by sqrt(2)) + L_s^T R_s (K=3 augment)

with L = [valid; 2*sq; valid], R = [-sq; -valid/2; valid] so that for a
real-real pair psum = 1 - d exactly, and any pair involving a padding
column gives psum = 0. A single ScalarE activation per PSUM bank then
computes relu(psum) and its per-partition running sum (accum_out), which
is the entire hinge contribution -- no masks, no pad corrections.

Device traffic per core: 1/8th of x in bf16 (~426 KB) + two 10 KB side
tensors; 26 small matmuls; 4 activations; no DVE work.
"""

import numpy as np
import ml_dtypes

N, D = 8192, 128
MARGIN = 1.0
NCORES = 8
SLOT = 128               # columns per class slot (max class size supported)
SLOTS_PER_CORE = 13      # 8*13 = 104 slots >= 100 classes
W = SLOTS_PER_CORE * SLOT
# bank groups of slots: each group's matmuls share one PSUM bank (<=512 free)
GROUPS = [(0, 4), (4, 4), (8, 4), (12, 1)]

_BF16 = ml_dtypes.bfloat16
_NC = None


# PSUM regions: region A covers slots 0..7 (2 banks), region B slots 8..12
NSLOTS_A = 8
NSLOTS_B = SLOTS_PER_CORE - NSLOTS_A


def _build_nc():
    """Raw bacc program (no TileContext): manual semaphores, minimal
    instruction count. The rank-3 correction matmuls depend only on the
    small l2/r2 DMA, so they issue first and double as PE warm-up while
    the big x1 DMA is still in flight."""
    import concourse.bacc as bacc
    import concourse.mybir as mybir

    nc = bacc.Bacc(None, target_bir_lowering=False)
    bf16 = mybir.dt.bfloat16
    f32 = mybir.dt.float32

    x1 = nc.declare_dram_parameter("x1", [D, W], bf16, isOutput=False)
    l2 = nc.declare_dram_parameter("l2", [3, W], bf16, isOutput=False)
    r2 = nc.declare_dram_parameter("r2", [3, W], bf16, isOutput=False)
    acc = nc.declare_dram_parameter("acc", [D, 2], f32, isOutput=True)

    with (
        nc.sbuf_tensor("x1t", [D, W], bf16) as x1t,
        nc.sbuf_tensor("l2t", [3, W], bf16) as l2t,
        nc.sbuf_tensor("r2t", [3, W], bf16) as r2t,
        nc.sbuf_tensor("accst", [D, 2], f32) as accst,
        nc.sbuf_tensor("v1a", [D, NSLOTS_A * SLOT], bf16) as v1a,
        nc.sbuf_tensor("v1b", [D, NSLOTS_B * SLOT], bf16) as v1b,
        nc.sbuf_tensor("zbias", [D, 1], f32) as zbias,
        nc.psum_tensor("psa", [D, NSLOTS_A * SLOT], f32) as psa,
        nc.psum_tensor("psb", [D, NSLOTS_B * SLOT], f32) as psb,
        nc.semaphore("s_lr") as s_lr,
        nc.semaphore("s_x1") as s_x1,
        nc.semaphore("s_mm") as s_mm,
        nc.semaphore("s_act") as s_act,
        nc.semaphore("s_z") as s_z,
        nc.semaphore("s_out") as s_out,
        nc.Block() as block,
    ):
        def pslot(s):
            if s < NSLOTS_A:
                return psa[:, s * SLOT : (s + 1) * SLOT]
            s -= NSLOTS_A
            return psb[:, s * SLOT : (s + 1) * SLOT]

        @block.sync
        def _(sync):
            # big tensor first on the Sync queue (direct DMAs serialize on
            # their issuing engine); completion covered by the exit barrier
            sync.dma_start(x1t[:], x1[:]).then_inc(s_x1, 16)
            sync.wait_ge(s_act, 2)
            sync.dma_start(acc[:], accst[:]).then_inc(s_out, 16)

        @block.gpsimd
        def _(gpsimd):
            # side tensors via the otherwise-idle GpSimd queue, concurrent
            # with the x1 DMA
            gpsimd.dma_start(l2t[:], l2[:]).then_inc(s_lr, 16)
            gpsimd.dma_start(r2t[:], r2[:]).then_inc(s_lr, 16)

        def mm1(s):
            # K=128 gram matmul opens slot s's accumulation group.
            # start=True marks the whole 2 KB PSUM bank pending-zero, so a
            # slot's group may only open after the previous slot in the same
            # bank has fully closed (bank-first slots: 0, 4, 8, 12).
            cols = slice(s * SLOT, (s + 1) * SLOT)
            return nc.tensor.matmul(
                pslot(s), x1t[:, cols], x1t[:, cols],
                start=True, stop=False,
            )

        def mm2(s):
            # rank-3 augmentation accumulates onto the gram part, closes slot
            cols = slice(s * SLOT, (s + 1) * SLOT)
            return nc.tensor.matmul(
                pslot(s), l2t[:, cols], r2t[:, cols],
                start=False, stop=True,
            )

        @block.tensor
        def _(tensor):
            tensor.wait_ge(s_x1, 16)
            # open the first slot of each psum bank; only needs x1
            for s in (0, 4, 8, 12):
                mm1(s)
            tensor.wait_ge(s_lr, 32)
            for s in range(SLOTS_PER_CORE):
                mm = mm2(s)
                if s in (NSLOTS_A - 1, SLOTS_PER_CORE - 1):
                    mm.then_inc(s_mm, 1)
                nxt = s + 1
                if nxt < SLOTS_PER_CORE and nxt not in (4, 8, 12):
                    mm1(nxt)

        @block.scalar
        def _(scalar):
            nc.scalar.memzero(zbias[:]).then_inc(s_z, 1)
            scalar.wait_ge(s_z, 1)
            scalar.wait_ge(s_mm, 1)
            nc.scalar.activation(
                v1a[:], psa[:], mybir.ActivationFunctionType.Relu,
                bias=zbias[:], scale=1.0, accum_out=accst[:, 0:1],
            ).then_inc(s_act, 1)
            scalar.wait_ge(s_mm, 2)
            nc.scalar.activation(
                v1b[:], psb[:], mybir.ActivationFunctionType.Relu,
                bias=zbias[:], scale=1.0, accum_out=accst[:, 1:2],
            ).then_inc(s_act, 1)

    nc.finalize()
    return nc


def _get_nc():
    global _NC
    if _NC is None:
        _NC = _build_nc()
    return _NC


def _prepare_inputs(x_np, y_np):
    """Host-side packing + exact fp64 moment sums.

    Returns (in_maps, sum_d_all, sum_d_same)."""
    x64 = x_np.astype(np.float64)
    sq64 = np.einsum("ij,ij->i", x64, x64)
    s_all = x64.sum(0)
    sum_d_all = 2.0 * N * sq64.sum() - 2.0 * float(s_all @ s_all)

    sq32 = sq64.astype(np.float32)
    order = np.argsort(y_np, kind="stable")
    uniq, counts = np.unique(y_np, return_counts=True)
    assert len(uniq) <= NCORES * SLOTS_PER_CORE, "too many classes for slots"
    assert counts.max() <= SLOT, "class larger than one slot"

    X1 = np.zeros((NCORES, D, W), np.float32)
    LR = np.zeros((NCORES, 6, W), np.float32)
    sum_d_same = 0.0
    pos = 0
    root2 = np.float32(np.sqrt(2.0))
    for ci, n_c in enumerate(counts):
        idx = order[pos : pos + n_c]
        pos += n_c
        core, ls = divmod(ci, SLOTS_PER_CORE)
        c0 = ls * SLOT
        X1[core][:, c0 : c0 + n_c] = (root2 * x_np[idx]).T
        # lhsT rows [valid; 2*sq; valid], rhs rows [-sq; -valid/2; valid]:
        # psum = 2 x.x - sq_i - sq_j + valid_i*valid_j = 1 - d for real
        # pairs, exactly 0 when either side is padding.
        LR[core][0, c0 : c0 + n_c] = 1.0
        LR[core][1, c0 : c0 + n_c] = 2.0 * sq32[idx]
        LR[core][2, c0 : c0 + n_c] = 1.0
        LR[core][3, c0 : c0 + n_c] = -sq32[idx]
        LR[core][4, c0 : c0 + n_c] = -0.5
        LR[core][5, c0 : c0 + n_c] = 1.0
        sc = x64[idx].sum(0)
        sum_d_same += 2.0 * n_c * sq64[idx].sum() - 2.0 * float(sc @ sc)

    in_maps = [
        {
            "x1": np.ascontiguousarray(X1[i]).astype(_BF16),
            "l2": np.ascontiguousarray(LR[i][0:3]).astype(_BF16),
            "r2": np.ascontiguousarray(LR[i][3:6]).astype(_BF16),
        }
        for i in range(NCORES)
    ]
    return in_maps, sum_d_all, sum_d_same


def _run_device(in_maps, trace=False):
    from concourse.bass_utils import run_bass_kernel_spmd

    return run_bass_kernel_spmd(
        _get_nc(), in_maps, core_ids=list(range(NCORES)), trace=trace
    )


def kernel(x, y):
    x_np = np.asarray(x, dtype=np.float32).reshape(N, D)
    y_np = np.asarray(y).astype(np.int64).ravel()

    in_maps, sum_d_all, sum_d_same = _prepare_inputs(x_np, y_np)
    res = _run_device(in_maps)
    hinge = sum(float(r["acc"].astype(np.float64).sum()) for r in res.results)

    loss = (sum_d_all - sum_d_same + hinge) / (float(N) * float(N))
    return np.float32(loss)


# revision 17
# speedup vs baseline: 1.0762x; 1.0174x over previous
"""Trainium2 Bass kernel for nn_ContrastiveEmbeddingLoss.

Reference computation (N=8192, D=128, margin=1.0):
    d[i,j]  = ||x_i - x_j||^2          (clamped at 0)
    same    = (y_i == y_j)
    loss    = mean((1-same)*d + same*relu(margin - d))

Algebraic decomposition used here:
    loss_sum = sum_ij d  -  sum_same d  +  sum_same relu(1 - d)

The first two terms are exact O(N*D) sums-of-moments:
    sum_ij d   = 2*N*sum(sq) - 2*||sum_i x_i||^2
    sum_same d = sum_c [ 2*n_c*sum_{i in c} sq_i - 2*||sum_{i in c} x_i||^2 ]
computed on host in float64 (tiny next to the device work and *more*
accurate than the reference's own fp32 mean over 67M elements).

Only sum_same relu(1 - d) needs pairwise work, and `same` is nonzero only
within a class. The host sorts rows by class into 128-wide slots
(13 slots/core x 8 cores = 104 >= number of classes), and each core runs:

    psum = X_s^T X_s (K=128, scaled by sqrt(2)) + L_s^T R_s (K=3 augment)

with L = [valid; 2*sq; valid], R = [-sq; -valid/2; valid] so that for a
real-real pair psum = 1 - d exactly, and any pair involving a padding
column gives psum = 0. A single ScalarE activation per PSUM bank then
computes relu(psum) and its per-partition running sum (accum_out), which
is the entire hinge contribution -- no masks, no pad corrections.

Device traffic per core: 1/8th of x in bf16 (~426 KB) + two 10 KB side
tensors; 26 small matmuls; 4 activations; no DVE work.
"""

import numpy as np
import ml_dtypes

N, D = 8192, 128
MARGIN = 1.0
NCORES = 8
SLOT = 128               # columns per class slot (max class size supported)
SLOTS_PER_CORE = 13      # 8*13 = 104 slots >= 100 classes
W = SLOTS_PER_CORE * SLOT
# bank groups of slots: each group's matmuls share one PSUM bank (<=512 free)
GROUPS = [(0, 4), (4, 4), (8, 4), (12, 1)]

_BF16 = ml_dtypes.bfloat16
_NC = None


# PSUM regions: region A covers slots 0..7 (2 banks), region B slots 8..12
NSLOTS_A = 8
NSLOTS_B = SLOTS_PER_CORE - NSLOTS_A


def _build_nc():
    """Raw bacc program (no TileContext): manual semaphores, minimal
    instruction count. The rank-3 correction matmuls depend only on the
    small l2/r2 DMA, so they issue first and double as PE warm-up while
    the big x1 DMA is still in flight."""
    import concourse.bacc as bacc
    import concourse.mybir as mybir

    nc = bacc.Bacc(None, target_bir_lowering=False)
    bf16 = mybir.dt.bfloat16
    f32 = mybir.dt.float32

    x1 = nc.declare_dram_parameter("x1", [D, W], bf16, isOutput=False)
    l2 = nc.declare_dram_parameter("l2", [3, W], bf16, isOutput=False)
    r2 = nc.declare_dram_parameter("r2", [3, W], bf16, isOutput=False)
    acc = nc.declare_dram_parameter("acc", [D, 2], f32, isOutput=True)

    with (
        nc.sbuf_tensor("x1t", [D, W], bf16) as x1t,
        nc.sbuf_tensor("l2t", [3, W], bf16) as l2t,
        nc.sbuf_tensor("r2t", [3, W], bf16) as r2t,
        nc.sbuf_tensor("accst", [D, 2], f32) as accst,
        nc.sbuf_tensor("v1a", [D, NSLOTS_A * SLOT], bf16) as v1a,
        nc.sbuf_tensor("v1b", [D, NSLOTS_B * SLOT], bf16) as v1b,
        nc.sbuf_tensor("zbias", [D, 1], f32) as zbias,
        nc.psum_tensor("psa", [D, NSLOTS_A * SLOT], f32) as psa,
        nc.psum_tensor("psb", [D, NSLOTS_B * SLOT], f32) as psb,
        nc.semaphore("s_lr") as s_lr,
        nc.semaphore("s_x1") as s_x1,
        nc.semaphore("s_mm") as s_mm,
        nc.semaphore("s_act") as s_act,
        nc.semaphore("s_z") as s_z,
        nc.semaphore("s_out") as s_out,
        nc.semaphore("s_c1") as s_c1,
        nc.semaphore("s_c2") as s_c2,
        nc.Block() as block,
    ):
        def pslot(s):
            if s < NSLOTS_A:
                return psa[:, s * SLOT : (s + 1) * SLOT]
            s -= NSLOTS_A
            return psb[:, s * SLOT : (s + 1) * SLOT]

        # x1 is split into three column chunks DMA'd concurrently from three
        # different engines (a single direct DMA moves ~130 GB/s; three
        # queues run in parallel). Chunk 0 carries slots 0-4 so the PE can
        # open banks as soon as it lands.
        C0, C1 = 5 * SLOT, 9 * SLOT  # chunk boundaries: slots 0-4, 5-8, 9-12

        @block.sync
        def _(sync):
            sync.dma_start(x1t[:, 0:C0], x1[:, 0:C0]).then_inc(s_x1, 16)
            sync.wait_ge(s_act, 2)
            sync.dma_start(acc[:], accst[:]).then_inc(s_out, 16)

        @block.gpsimd
        def _(gpsimd):
            gpsimd.dma_start(x1t[:, C1:W], x1[:, C1:W]).then_inc(s_c2, 16)
            gpsimd.dma_start(l2t[:], l2[:]).then_inc(s_lr, 16)
            gpsimd.dma_start(r2t[:], r2[:]).then_inc(s_lr, 16)

        def mm1(s):
            # K=128 gram matmul opens slot s's accumulation group.
            # start=True marks the whole 2 KB PSUM bank pending-zero, so a
            # slot's group may only open after the previous slot in the same
            # bank has fully closed (bank-first slots: 0, 4, 8, 12).
            cols = slice(s * SLOT, (s + 1) * SLOT)
            return nc.tensor.matmul(
                pslot(s), x1t[:, cols], x1t[:, cols],
                start=True, stop=False,
            )

        def mm2(s):
            # rank-3 augmentation accumulates onto the gram part, closes slot
            cols = slice(s * SLOT, (s + 1) * SLOT)
            return nc.tensor.matmul(
                pslot(s), l2t[:, cols], r2t[:, cols],
                start=False, stop=True,
            )

        @block.tensor
        def _(tensor):
            # open the first slot of each psum bank as its chunk arrives
            tensor.wait_ge(s_x1, 16)
            mm1(0)
            mm1(4)
            tensor.wait_ge(s_c1, 16)
            mm1(8)
            tensor.wait_ge(s_c2, 16)
            mm1(12)
            tensor.wait_ge(s_lr, 32)
            for s in range(SLOTS_PER_CORE):
                mm = mm2(s)
                if s in (NSLOTS_A - 1, SLOTS_PER_CORE - 1):
                    mm.then_inc(s_mm, 1)
                nxt = s + 1
                if nxt < SLOTS_PER_CORE and nxt not in (4, 8, 12):
                    mm1(nxt)

        @block.scalar
        def _(scalar):
            scalar.dma_start(x1t[:, C0:C1], x1[:, C0:C1]).then_inc(s_c1, 16)
            nc.scalar.memzero(zbias[:]).then_inc(s_z, 1)
            scalar.wait_ge(s_z, 1)
            scalar.wait_ge(s_mm, 1)
            nc.scalar.activation(
                v1a[:], psa[:], mybir.ActivationFunctionType.Relu,
                bias=zbias[:], scale=1.0, accum_out=accst[:, 0:1],
            ).then_inc(s_act, 1)
            scalar.wait_ge(s_mm, 2)
            nc.scalar.activation(
                v1b[:], psb[:], mybir.ActivationFunctionType.Relu,
                bias=zbias[:], scale=1.0, accum_out=accst[:, 1:2],
            ).then_inc(s_act, 1)

    nc.finalize()
    return nc


def _get_nc():
    global _NC
    if _NC is None:
        _NC = _build_nc()
    return _NC


def _prepare_inputs(x_np, y_np):
    """Host-side packing + exact fp64 moment sums.

    Returns (in_maps, sum_d_all, sum_d_same)."""
    x64 = x_np.astype(np.float64)
    sq64 = np.einsum("ij,ij->i", x64, x64)
    s_all = x64.sum(0)
    sum_d_all = 2.0 * N * sq64.sum() - 2.0 * float(s_all @ s_all)

    sq32 = sq64.astype(np.float32)
    order = np.argsort(y_np, kind="stable")
    uniq, counts = np.unique(y_np, return_counts=True)
    assert len(uniq) <= NCORES * SLOTS_PER_CORE, "too many classes for slots"
    assert counts.max() <= SLOT, "class larger than one slot"

    X1 = np.zeros((NCORES, D, W), np.float32)
    LR = np.zeros((NCORES, 6, W), np.float32)
    sum_d_same = 0.0
    pos = 0
    root2 = np.float32(np.sqrt(2.0))
    for ci, n_c in enumerate(counts):
        idx = order[pos : pos + n_c]
        pos += n_c
        core, ls = divmod(ci, SLOTS_PER_CORE)
        c0 = ls * SLOT
        X1[core][:, c0 : c0 + n_c] = (root2 * x_np[idx]).T
        # lhsT rows [valid; 2*sq; valid], rhs rows [-sq; -valid/2; valid]:
        # psum = 2 x.x - sq_i - sq_j + valid_i*valid_j = 1 - d for real
        # pairs, exactly 0 when either side is padding.
        LR[core][0, c0 : c0 + n_c] = 1.0
        LR[core][1, c0 : c0 + n_c] = 2.0 * sq32[idx]
        LR[core][2, c0 : c0 + n_c] = 1.0
        LR[core][3, c0 : c0 + n_c] = -sq32[idx]
        LR[core][4, c0 : c0 + n_c] = -0.5
        LR[core][5, c0 : c0 + n_c] = 1.0
        sc = x64[idx].sum(0)
        sum_d_same += 2.0 * n_c * sq64[idx].sum() - 2.0 * float(sc @ sc)

    in_maps = [
        {
            "x1": np.ascontiguousarray(X1[i]).astype(_BF16),
            "l2": np.ascontiguousarray(LR[i][0:3]).astype(_BF16),
            "r2": np.ascontiguousarray(LR[i][3:6]).astype(_BF16),
        }
        for i in range(NCORES)
    ]
    return in_maps, sum_d_all, sum_d_same


def _run_device(in_maps, trace=False):
    from concourse.bass_utils import run_bass_kernel_spmd

    return run_bass_kernel_spmd(
        _get_nc(), in_maps, core_ids=list(range(NCORES)), trace=trace
    )


def kernel(x, y):
    x_np = np.asarray(x, dtype=np.float32).reshape(N, D)
    y_np = np.asarray(y).astype(np.int64).ravel()

    in_maps, sum_d_all, sum_d_same = _prepare_inputs(x_np, y_np)
    res = _run_device(in_maps)
    hinge = sum(float(r["acc"].astype(np.float64).sum()) for r in res.results)

    loss = (sum_d_all - sum_d_same + hinge) / (float(N) * float(N))
    return np.float32(loss)
